# revision 47
# baseline (speedup 1.0000x reference)
"""LiteMLA (linear attention) Trainium2 kernel, v2.

Full-input contract: kernel(**inputs) takes the unsharded tensors from
setup_inputs() and returns the full (16, 256, 64, 64) float32 output.

Strategy
--------
Data-parallel over batch: 16 batch elements -> 8 NeuronCores x 2 each.
Weights replicated. Per batch element (C=256, N=4096):

  q  = relu(Wq x)                  (C, N)   Wq^T stationary
  kT = relu((Wk x)^T)              (N, C)   x chunks stationary
  GT[c',c] = sum_n xT[n,c'] kT[n,c]        (C, C) PSUM accumulation
       xT comes PRE-TRANSPOSED from the host (no PE transposes).
  ksum[c]  = sum_n kT[n,c]                 via rhs=ones column, ap=1
  MT = GT^T-contract with Wvp              Wvp = Wv^T (diag(scale) Wproj)^T
       folds the v-projection AND the output projection AND the BN scale
       into the tiny rank-C state (host-precomputed Wvp).
  mt2 = MT + ksum ⊗ bias'                  folds the BN bias through the
       normalizer:  (z + bias*den)/den = z/den + bias
  den[p,n] = sum_c ksum[c] q[c,n]          broadcast-by-matmul trick
  y = (mt2^T q) * (1/den)                  single DVE multiply per tile

All matmuls bf16 (fp32 PSUM accum); eps is negligible vs den~O(1e4).
"""

import numpy as np
import ml_dtypes

import concourse.bass as bass
from concourse import bacc
import concourse.mybir as mybir
import concourse.tile as tile
from concourse.bass_utils import run_bass_kernel_spmd

B, C, H, W = 16, 256, 64, 64
N = H * W            # 4096
NCORES = 8
BL = B // NCORES     # batch elements per core
NT = N // 128        # 32 n-subtiles for kT/GT
NTW = N // 512       # 8 wide n-tiles for q/z/den
BF16 = mybir.dt.bfloat16
F32 = mybir.dt.float32
NPBF16 = ml_dtypes.bfloat16

# const-pack column offsets (bf16 [128, CONSTW]); wk + onec first so the
# cold-start kT/ksum path only waits on the first slice of the const DMA
OFF_WK = 0            # 2 x 256
OFF_ONEC = 512        # 1     ones column
OFF_WQ = 513          # 2 x 256
OFF_WVP = 1025        # 2 x 256
OFF_ONES = 1537       # 128   ones
CONSTW = 1665
CST_SPLIT = 513       # first const DMA covers wk + onec

_CACHE = {}


def _build_program():
    nc = bacc.Bacc("TRN2", target_bir_lowering=False, debug=False)

    xs = nc.dram_tensor("x", [BL, 2, 128, N], BF16, kind="ExternalInput")
    xts = nc.dram_tensor("xt", [BL, 128, NT * C], BF16, kind="ExternalInput")
    cst = nc.dram_tensor("cst", [128, CONSTW], BF16, kind="ExternalInput")
    bc = nc.dram_tensor("bcol", [C, 1], F32, kind="ExternalInput")
    ys = nc.dram_tensor("y", [BL, C, N], F32, kind="ExternalOutput")

    Relu = mybir.ActivationFunctionType.Relu
    Ident = mybir.ActivationFunctionType.Identity
    Mult = mybir.AluOpType.mult

    with tile.TileContext(nc) as tc:
        with (
            tc.tile_pool(name="const", bufs=1) as cp,
            tc.tile_pool(name="xp", bufs=2) as xp,
            tc.tile_pool(name="xtp", bufs=2) as xtp,
            tc.tile_pool(name="qp", bufs=2) as qp,
            tc.tile_pool(name="ktp", bufs=3) as ktp,
            tc.tile_pool(name="small", bufs=2) as sp,
            tc.tile_pool(name="yst", bufs=2) as yp,
            tc.tile_pool(name="ps_kt", bufs=2, space="PSUM") as ps_kt,
            tc.tile_pool(name="ps_gt", bufs=1, space="PSUM") as ps_gt,
            tc.tile_pool(name="ps_ks", bufs=1, space="PSUM") as ps_ks,
            tc.tile_pool(name="ps_w", bufs=4, space="PSUM") as ps_w,
        ):
            # ---------- cold-start DMA order: slices sequenced to track PE
            # consumption (pair p eats x cols 256p:256p+256, GT eats xt cols
            # 512(p-1):512p) ----------
            x_all = [[xp.tile([128, N], BF16, tag=f"x{k}", name=f"x{k}_{b}") for k in range(2)]
                     for b in range(BL)]
            xt_all = [xtp.tile([128, NT * C], BF16, tag="xt", name=f"xt_{b}")
                      for b in range(BL)]

            # loads are spread across DMA-capable queues: x-k0 + xt-low on SP,
            # x-k1 + xt-high on Pool — each queue serializes its own transfers.
            def ld_x(b, k, c0, c1):
                eng = nc.sync if k == 0 else nc.gpsimd
                eng.dma_start(out=x_all[b][k][:, c0:c1], in_=xs[b, k, :, c0:c1])

            def ld_xt(b, c0, c1, eng=None):
                (eng or nc.sync).dma_start(out=xt_all[b][:, c0:c1], in_=xts[b, :, c0:c1])

            ld_x(0, 0, 0, 512)
            ld_x(0, 1, 0, 512)
            cst_sb = cp.tile([128, CONSTW], BF16, tag="cst", name="cst")
            nc.sync.dma_start(out=cst_sb[:, 0:CST_SPLIT], in_=cst[:, 0:CST_SPLIT])
            ld_x(0, 1, 512, 2048)
            ld_xt(0, 0, 1024)
            ld_x(0, 0, 512, 2048)
            ld_xt(0, 4096, 6144, nc.gpsimd)
            ld_xt(0, 1024, 2048)
            ld_x(0, 1, 2048, 4096)
            ld_xt(0, 2048, 4096)
            ld_xt(0, 6144, 8192, nc.gpsimd)
            ld_x(0, 0, 2048, 4096)
            nc.gpsimd.dma_start(out=cst_sb[:, CST_SPLIT:CONSTW], in_=cst[:, CST_SPLIT:CONSTW])
            wq = [cst_sb[:, OFF_WQ + k * 256:OFF_WQ + (k + 1) * 256] for k in range(2)]
            wk = [cst_sb[:, OFF_WK + k * 256:OFF_WK + (k + 1) * 256] for k in range(2)]
            wvp = [cst_sb[:, OFF_WVP + k * 256:OFF_WVP + (k + 1) * 256] for k in range(2)]
            ones128 = cst_sb[:, OFF_ONES:OFF_ONES + 128]
            onec = cst_sb[:, OFF_ONEC:OFF_ONEC + 1]
            bias_sb = [cp.tile([128, 1], F32, tag=f"bias{oc}", name=f"bias{oc}") for oc in range(2)]
            for oc in range(2):
                nc.gpsimd.dma_start(out=bias_sb[oc][:], in_=bc[oc * 128:(oc + 1) * 128, :])

            # q_mms emits one n-tile of the q matmuls; phase2_iw one n-tile of
            # den/z/y. unit() skews them (q at iw, phase2 at iw-1) so the ACT
            # relu latency is hidden, and units are interleaved into the NEXT
            # batch's phase-1 so PE never drains.
            def q_mms(st, iw):
                b, q_sb = st["b"], st["q"]
                nsl = slice(iw * 512, (iw + 1) * 512)
                x_b = x_all[b]
                for mc in range(2):
                    qps = ps_w.tile([128, 512], F32, tag="w", name=f"qps_{b}_{iw}_{mc}")
                    for k in range(2):
                        nc.tensor.matmul(qps[:], lhsT=wq[k][:, mc * 128:(mc + 1) * 128],
                                         rhs=x_b[k][:, nsl], start=(k == 0), stop=(k == 1))
                    nc.scalar.activation(q_sb[mc][:, nsl], qps[:], Relu)

            def phase2_iw(st, iw):
                b, q_sb, mt_sb, u_sb = st["b"], st["q"], st["mt"], st["u"]
                nsl = slice(iw * 512, (iw + 1) * 512)
                dbps = ps_w.tile([128, 512], F32, tag="w", name=f"dbps_{b}_{iw}")
                for cs in range(2):
                    nc.tensor.matmul(dbps[:], lhsT=u_sb[cs][:], rhs=q_sb[cs][:, nsl],
                                     start=(cs == 0), stop=(cs == 1))
                s_sb = sp.tile([128, 512], F32, tag="s", name=f"s_{b}_{iw}")
                nc.vector.reciprocal_approx_fast(s_sb[:], dbps[:])
                drain = st.get("drain", False)
                if not drain and iw % 2 == 0:
                    st["yst"] = [yp.tile([128, 1024], F32, tag=f"y{oc}", name=f"y{oc}_{b}_{iw}")
                                 for oc in range(2)]
                yst = st["yst"]
                for oc in range(2):
                    zps = ps_w.tile([128, 512], F32, tag="w", name=f"zps_{b}_{iw}_{oc}")
                    for cs in range(2):
                        nc.tensor.matmul(zps[:],
                                         lhsT=mt_sb[cs][:, oc * 128:(oc + 1) * 128],
                                         rhs=q_sb[cs][:, nsl],
                                         start=(cs == 0), stop=(cs == 1))
                    t_sb = sp.tile([128, 512], F32, tag=f"t{oc}", name=f"t_{b}_{iw}_{oc}")
                    nc.vector.tensor_tensor(t_sb[:], zps[:], s_sb[:], Mult)
                    if drain:
                        # drain: 512-wide immediate stores, chain spread over
                        # ACT/Pool bias + ACT/SP store queues to shorten the tail
                        yd = yp.tile([128, 512], F32, tag=f"yd{oc}", name=f"yd{oc}_{b}_{iw}")
                        if oc == 0:
                            nc.scalar.activation(yd[:], t_sb[:], Ident, bias=bias_sb[oc][:])
                            seng = nc.scalar if iw >= 6 else nc.gpsimd
                        else:
                            nc.gpsimd.tensor_scalar_add(yd[:], t_sb[:], bias_sb[oc][:])
                            seng = nc.sync
                        seng.dma_start(out=ys[b, oc * 128:(oc + 1) * 128, nsl], in_=yd[:])
                    else:
                        ysl = yst[oc][:, (iw % 2) * 512:(iw % 2 + 1) * 512]
                        nc.gpsimd.tensor_scalar_add(ysl, t_sb[:], bias_sb[oc][:])
                if not drain and iw % 2 == 1:
                    for oc in range(2):
                        # spread store issue across the SP and Pool queues
                        eng = nc.gpsimd if oc == 0 else nc.sync
                        eng.dma_start(
                            out=ys[b, oc * 128:(oc + 1) * 128, (iw - 1) * 512:(iw + 1) * 512],
                            in_=yst[oc][:])

            def unit(st, j):
                # slot j runs 0..NTW-1: q tile j+1 (skew 1), then phase2 for j.
                # q tile 0 is emitted at the owning batch's section end,
                # filling the PE bubble while DVE copies gt/ks out of PSUM.
                if j + 1 < NTW:
                    q_mms(st, j + 1)
                phase2_iw(st, j)

            prev = None
            for b in range(BL):
                # ---------- load x + xT (batch 0 pre-issued above) ----------
                x_sb = x_all[b]
                xt_sb = xt_all[b]
                if b > 0:
                    ld_x(b, 0, 0, 2048)
                    ld_x(b, 1, 0, 2048)
                    ld_xt(b, 0, 4096)
                    ld_xt(b, 4096, 8192, nc.gpsimd)
                    ld_x(b, 0, 2048, N)
                    ld_x(b, 1, 2048, N)

                # ---------- phase 1: kT, GT, ksum (software-pipelined), with
                # the previous batch's phase 2 interleaved ----------
                # PSUM zero-region rule: start=True marks the whole 2KB bank
                # pending-zero, so per shared bank only the FIRST accumulation
                # group may start; later groups ride the pending flag.
                gt_ps = ps_gt.tile([128, 512], F32, tag="gt", name=f"gt_{b}")
                ks_ps = ps_ks.tile([128, 512], F32, tag="ks", name=f"ks_{b}")

                def gt_ksum_mms(kt, jpair):
                    for half in range(2):
                        j = 2 * jpair + half
                        ktv = kt[:, half * 256:(half + 1) * 256]
                        for mc in range(2):
                            nc.tensor.matmul(gt_ps[:, mc * 256:(mc + 1) * 256],
                                             lhsT=xt_sb[:, j * 256 + mc * 128:j * 256 + (mc + 1) * 128],
                                             rhs=ktv, start=(j == 0 and mc == 0),
                                             stop=(j == NT - 1),
                                             skip_group_check=True)
                        for cc in range(2):
                            nc.tensor.matmul(ks_ps[:, cc:cc + 1],
                                             lhsT=ktv[:, cc * 128:(cc + 1) * 128],
                                             rhs=onec, start=(j == 0 and cc == 0),
                                             stop=(j == NT - 1),
                                             skip_group_check=True)

                kt_prev = None
                for p in range(NT // 2):
                    ktps = ps_kt.tile([128, 512], F32, tag="kt", name=f"ktps_{b}_{p}")
                    for half in range(2):
                        i = 2 * p + half
                        for k in range(2):
                            nc.tensor.matmul(ktps[:, half * 256:(half + 1) * 256],
                                             lhsT=x_sb[k][:, i * 128:(i + 1) * 128],
                                             rhs=wk[k], start=(half == 0 and k == 0),
                                             stop=(half == 1 and k == 1),
                                             skip_group_check=True)
                    if kt_prev is not None:
                        gt_ksum_mms(*kt_prev)
                    if prev is not None and p % 2 == 1:
                        unit(prev, p // 2)
                    kt_sb = ktp.tile([128, 512], BF16, tag="kt_sb", name=f"kt_sb_{b}_{p}")
                    nc.scalar.activation(kt_sb[:], ktps[:], Relu)
                    kt_prev = (kt_sb, p)
                gt_ksum_mms(*kt_prev)

                # ---------- tail state (DVE) + q tile 0 (fills PE bubble) ----------
                gt_sb = sp.tile([128, 512], BF16, tag="gt_sb", name=f"gt_sb_{b}")
                nc.vector.tensor_copy(gt_sb[:], gt_ps[:])
                ks_sb = sp.tile([128, 2], F32, tag="ks_sb", name=f"ks_sb_{b}")
                nc.vector.tensor_copy(ks_sb[:], ks_ps[:, 0:2])

                q_sb = [qp.tile([128, N], BF16, tag=f"q{mc}", name=f"q{mc}_{b}") for mc in range(2)]
                st_b = {"b": b, "q": q_sb, "yst": None}
                q_mms(st_b, 0)

                # ---------- MT fold + U ----------
                mt_sb = []
                for cs in range(2):
                    mtps = ps_w.tile([128, 512], F32, tag="w", name=f"mtps_{b}_{cs}")
                    for mc in range(2):
                        nc.tensor.matmul(mtps[:, 0:256],
                                         lhsT=gt_sb[:, mc * 256 + cs * 128:mc * 256 + (cs + 1) * 128],
                                         rhs=wvp[mc], start=(mc == 0), stop=(mc == 1))
                    m = sp.tile([128, 256], BF16, tag=f"mt_{cs}", name=f"mt_{b}_{cs}")
                    nc.vector.tensor_copy(m[:], mtps[:, 0:256])
                    mt_sb.append(m)
                u_sb = []
                for cs in range(2):
                    u = sp.tile([128, 128], BF16, tag=f"u_{cs}", name=f"u_{b}_{cs}")
                    nc.vector.tensor_scalar_mul(u[:], ones128, ks_sb[:, cs:cs + 1])
                    u_sb.append(u)

                st_b.update({"mt": mt_sb, "u": u_sb})
                prev = st_b

            # ---------- drain: last batch's q + phase 2, skewed ----------
            prev["drain"] = True
            for j in range(NTW):
                unit(prev, j)
    nc.compile()
    return nc


def _prep_inputs(x, w_qkv, w_proj, bn_gamma, bn_beta, bn_mean, bn_var):
    x = np.asarray(x, dtype=np.float32)
    w_qkv = np.asarray(w_qkv, dtype=np.float32)
    w_proj = np.asarray(w_proj, dtype=np.float32)
    bn_gamma = np.asarray(bn_gamma, dtype=np.float32)
    bn_beta = np.asarray(bn_beta, dtype=np.float32)
    bn_mean = np.asarray(bn_mean, dtype=np.float32)
    bn_var = np.asarray(bn_var, dtype=np.float32)

    # torch-faithful interleave: out-channel 3*i+j -> (channel i, {q,k,v}[j])
    wq_t = w_qkv[0::3].T          # [c_in, c_out]
    wk_t = w_qkv[1::3].T
    wv = w_qkv[2::3]              # [u, c']
    scale = bn_gamma / np.sqrt(bn_var + 1e-5)
    wp_s = scale[:, None] * w_proj            # Wp' [o, u]
    wvp = wv.T @ wp_s.T                       # [c', o]
    bias = bn_beta - bn_mean * scale          # [o]

    cstp = np.zeros((128, CONSTW), dtype=np.float32)
    for k in range(2):
        cstp[:, OFF_WQ + k * 256:OFF_WQ + (k + 1) * 256] = wq_t[k * 128:(k + 1) * 128]
        cstp[:, OFF_WK + k * 256:OFF_WK + (k + 1) * 256] = wk_t[k * 128:(k + 1) * 128]
        cstp[:, OFF_WVP + k * 256:OFF_WVP + (k + 1) * 256] = wvp[k * 128:(k + 1) * 128]
    cstp[:, OFF_ONES:OFF_ONES + 128] = 1.0
    cstp[:, OFF_ONEC] = 1.0
    cst_bf = np.ascontiguousarray(cstp.astype(NPBF16))
    bcol = np.ascontiguousarray(bias.astype(np.float32).reshape(C, 1))

    xf = x.reshape(B, C, N)
    x_bf = np.ascontiguousarray(xf.reshape(B, 2, 128, N).astype(NPBF16))
    # xt[b, p, i*256 + c] = xf[b, c, i*128 + p]
    xt_bf = np.ascontiguousarray(
        xf.reshape(B, C, NT, 128).transpose(0, 3, 2, 1).reshape(B, 128, NT * C)
        .astype(NPBF16))

    in_maps = []
    for core in range(NCORES):
        in_maps.append({
            "x": x_bf[core * BL:(core + 1) * BL],
            "xt": xt_bf[core * BL:(core + 1) * BL],
            "cst": cst_bf,
            "bcol": bcol,
        })
    return in_maps


def _run(inputs, trace=False, **kw):
    if "nc" not in _CACHE:
        _CACHE["nc"] = _build_program()
    nc = _CACHE["nc"]
    in_maps = _prep_inputs(**inputs)
    res = run_bass_kernel_spmd(nc, in_maps, list(range(NCORES)), trace=trace, **kw)
    y = np.concatenate([res.results[i]["y"] for i in range(NCORES)], axis=0)
    return y.reshape(B, C, H, W).astype(np.float32), res


def kernel(**inputs):
    y, _ = _run(inputs)
    return y


# revision 52
# speedup vs baseline: 1.0037x; 1.0037x over previous
"""LiteMLA (linear attention) Trainium2 kernel, v2.

Full-input contract: kernel(**inputs) takes the unsharded tensors from
setup_inputs() and returns the full (16, 256, 64, 64) float32 output.

Strategy
--------
Data-parallel over batch: 16 batch elements -> 8 NeuronCores x 2 each.
Weights replicated. Per batch element (C=256, N=4096):

  q  = relu(Wq x)                  (C, N)   Wq^T stationary
  kT = relu((Wk x)^T)              (N, C)   x chunks stationary
  GT[c',c] = sum_n xT[n,c'] kT[n,c]        (C, C) PSUM accumulation
       xT comes PRE-TRANSPOSED from the host (no PE transposes).
  ksum[c]  = sum_n kT[n,c]                 via rhs=ones column, ap=1
  MT = GT^T-contract with Wvp              Wvp = Wv^T (diag(scale) Wproj)^T
       folds the v-projection AND the output projection AND the BN scale
       into the tiny rank-C state (host-precomputed Wvp).
  mt2 = MT + ksum ⊗ bias'                  folds the BN bias through the
       normalizer:  (z + bias*den)/den = z/den + bias
  den[p,n] = sum_c ksum[c] q[c,n]          broadcast-by-matmul trick
  y = (mt2^T q) * (1/den)                  single DVE multiply per tile

All matmuls bf16 (fp32 PSUM accum); eps is negligible vs den~O(1e4).
"""

import numpy as np
import ml_dtypes

import concourse.bass as bass
from concourse import bacc
import concourse.mybir as mybir
import concourse.tile as tile
from concourse.bass_utils import run_bass_kernel_spmd

B, C, H, W = 16, 256, 64, 64
N = H * W            # 4096
NCORES = 8
BL = B // NCORES     # batch elements per core
NT = N // 128        # 32 n-subtiles for kT/GT
NTW = N // 512       # 8 wide n-tiles for q/z/den
BF16 = mybir.dt.bfloat16
F32 = mybir.dt.float32
NPBF16 = ml_dtypes.bfloat16

# const-pack column offsets (bf16 [128, CONSTW]); wk + onec first so the
# cold-start kT/ksum path only waits on the first slice of the const DMA
OFF_WK = 0            # 2 x 256
OFF_ONEC = 512        # 1     ones column
OFF_WQ = 513          # 2 x 256
OFF_WVP = 1025        # 2 x 256
OFF_ONES = 1537       # 128   ones
CONSTW = 1665
CST_SPLIT = 513       # first const DMA covers wk + onec

_CACHE = {}


def _build_program():
    nc = bacc.Bacc("TRN2", target_bir_lowering=False, debug=False)

    xs = nc.dram_tensor("x", [BL, 2, 128, N], BF16, kind="ExternalInput")
    xts = nc.dram_tensor("xt", [BL, 128, NT * C], BF16, kind="ExternalInput")
    cst = nc.dram_tensor("cst", [128, CONSTW], BF16, kind="ExternalInput")
    bc = nc.dram_tensor("bcol", [C, 1], F32, kind="ExternalInput")
    ys = nc.dram_tensor("y", [BL, C, N], F32, kind="ExternalOutput")

    Relu = mybir.ActivationFunctionType.Relu
    Ident = mybir.ActivationFunctionType.Identity
    Mult = mybir.AluOpType.mult

    with tile.TileContext(nc) as tc:
        with (
            tc.tile_pool(name="const", bufs=1) as cp,
            tc.tile_pool(name="xp", bufs=2) as xp,
            tc.tile_pool(name="xtp", bufs=2) as xtp,
            tc.tile_pool(name="qp", bufs=2) as qp,
            tc.tile_pool(name="ktp", bufs=3) as ktp,
            tc.tile_pool(name="small", bufs=2) as sp,
            tc.tile_pool(name="yst", bufs=2) as yp,
            tc.tile_pool(name="ps_kt", bufs=2, space="PSUM") as ps_kt,
            tc.tile_pool(name="ps_gt", bufs=1, space="PSUM") as ps_gt,
            tc.tile_pool(name="ps_ks", bufs=1, space="PSUM") as ps_ks,
            tc.tile_pool(name="ps_w", bufs=4, space="PSUM") as ps_w,
        ):
            # ---------- cold-start DMA order: slices sequenced to track PE
            # consumption (pair p eats x cols 256p:256p+256, GT eats xt cols
            # 512(p-1):512p) ----------
            x_all = [[xp.tile([128, N], BF16, tag=f"x{k}", name=f"x{k}_{b}") for k in range(2)]
                     for b in range(BL)]
            xt_all = [xtp.tile([128, NT * C], BF16, tag="xt", name=f"xt_{b}")
                      for b in range(BL)]

            # loads are spread across DMA-capable queues: x-k0 + xt-low on SP,
            # x-k1 + xt-high on Pool — each queue serializes its own transfers.
            def ld_x(b, k, c0, c1):
                eng = nc.sync if k == 0 else nc.gpsimd
                eng.dma_start(out=x_all[b][k][:, c0:c1], in_=xs[b, k, :, c0:c1])

            def ld_xt(b, c0, c1, eng=None):
                (eng or nc.sync).dma_start(out=xt_all[b][:, c0:c1], in_=xts[b, :, c0:c1])

            ld_x(0, 0, 0, 512)
            ld_x(0, 1, 0, 512)
            cst_sb = cp.tile([128, CONSTW], BF16, tag="cst", name="cst")
            nc.sync.dma_start(out=cst_sb[:, 0:CST_SPLIT], in_=cst[:, 0:CST_SPLIT])
            ld_x(0, 1, 512, 2048)
            ld_xt(0, 0, 1024)
            ld_x(0, 0, 512, 2048)
            ld_xt(0, 4096, 6144, nc.gpsimd)
            ld_xt(0, 1024, 2048)
            ld_x(0, 1, 2048, 4096)
            ld_xt(0, 2048, 4096)
            ld_xt(0, 6144, 8192, nc.gpsimd)
            ld_x(0, 0, 2048, 4096)
            nc.gpsimd.dma_start(out=cst_sb[:, CST_SPLIT:CONSTW], in_=cst[:, CST_SPLIT:CONSTW])
            wq = [cst_sb[:, OFF_WQ + k * 256:OFF_WQ + (k + 1) * 256] for k in range(2)]
            wk = [cst_sb[:, OFF_WK + k * 256:OFF_WK + (k + 1) * 256] for k in range(2)]
            wvp = [cst_sb[:, OFF_WVP + k * 256:OFF_WVP + (k + 1) * 256] for k in range(2)]
            ones128 = cst_sb[:, OFF_ONES:OFF_ONES + 128]
            onec = cst_sb[:, OFF_ONEC:OFF_ONEC + 1]
            bias_sb = [cp.tile([128, 1], F32, tag=f"bias{oc}", name=f"bias{oc}") for oc in range(2)]
            for oc in range(2):
                nc.gpsimd.dma_start(out=bias_sb[oc][:], in_=bc[oc * 128:(oc + 1) * 128, :])

            # q_mms emits one n-tile of the q matmuls; phase2_iw one n-tile of
            # den/z/y. unit() skews them (q at iw, phase2 at iw-1) so the ACT
            # relu latency is hidden, and units are interleaved into the NEXT
            # batch's phase-1 so PE never drains.
            def q_mms(st, iw):
                b, q_sb = st["b"], st["q"]
                nsl = slice(iw * 512, (iw + 1) * 512)
                x_b = x_all[b]
                for mc in range(2):
                    qps = ps_w.tile([128, 512], F32, tag="w", name=f"qps_{b}_{iw}_{mc}")
                    for k in range(2):
                        nc.tensor.matmul(qps[:], lhsT=wq[k][:, mc * 128:(mc + 1) * 128],
                                         rhs=x_b[k][:, nsl], start=(k == 0), stop=(k == 1))
                    nc.scalar.activation(q_sb[mc][:, nsl], qps[:], Relu)

            def phase2_iw(st, iw):
                b, q_sb, mt_sb, u_sb = st["b"], st["q"], st["mt"], st["u"]
                nsl = slice(iw * 512, (iw + 1) * 512)
                dbps = ps_w.tile([128, 512], F32, tag="w", name=f"dbps_{b}_{iw}")
                for cs in range(2):
                    nc.tensor.matmul(dbps[:], lhsT=u_sb[cs][:], rhs=q_sb[cs][:, nsl],
                                     start=(cs == 0), stop=(cs == 1))
                s_sb = sp.tile([128, 512], F32, tag="s", name=f"s_{b}_{iw}")
                nc.vector.reciprocal_approx_fast(s_sb[:], dbps[:])
                drain = st.get("drain", False)
                if not drain and iw % 2 == 0:
                    st["yst"] = [yp.tile([128, 1024], F32, tag=f"y{oc}", name=f"y{oc}_{b}_{iw}")
                                 for oc in range(2)]
                yst = st["yst"]
                for oc in range(2):
                    zps = ps_w.tile([128, 512], F32, tag="w", name=f"zps_{b}_{iw}_{oc}")
                    for cs in range(2):
                        nc.tensor.matmul(zps[:],
                                         lhsT=mt_sb[cs][:, oc * 128:(oc + 1) * 128],
                                         rhs=q_sb[cs][:, nsl],
                                         start=(cs == 0), stop=(cs == 1))
                    t_sb = sp.tile([128, 512], F32, tag=f"t{oc}", name=f"t_{b}_{iw}_{oc}")
                    nc.vector.tensor_tensor(t_sb[:], zps[:], s_sb[:], Mult)
                    if drain:
                        # drain: 512-wide immediate stores, chain spread over
                        # ACT/Pool bias + ACT/SP store queues to shorten the tail
                        yd = yp.tile([128, 512], F32, tag=f"yd{oc}", name=f"yd{oc}_{b}_{iw}")
                        if oc == 0:
                            nc.scalar.activation(yd[:], t_sb[:], Ident, bias=bias_sb[oc][:])
                            seng = nc.scalar if iw >= 6 else nc.gpsimd
                        else:
                            nc.gpsimd.tensor_scalar_add(yd[:], t_sb[:], bias_sb[oc][:])
                            seng = nc.sync
                        seng.dma_start(out=ys[b, oc * 128:(oc + 1) * 128, nsl], in_=yd[:])
                    else:
                        ysl = yst[oc][:, (iw % 2) * 512:(iw % 2 + 1) * 512]
                        nc.gpsimd.tensor_scalar_add(ysl, t_sb[:], bias_sb[oc][:])
                if not drain and iw % 2 == 1:
                    for oc in range(2):
                        # spread store issue across the SP and Pool queues
                        eng = nc.gpsimd if oc == 0 else nc.sync
                        eng.dma_start(
                            out=ys[b, oc * 128:(oc + 1) * 128, (iw - 1) * 512:(iw + 1) * 512],
                            in_=yst[oc][:])

            def unit(st, j):
                # slot j runs 0..NTW-1: q tile j+1 (skew 1), then phase2 for j.
                # q tile 0 is emitted at the owning batch's section end,
                # filling the PE bubble while DVE copies gt/ks out of PSUM.
                if j + 1 < NTW:
                    q_mms(st, j + 1)
                phase2_iw(st, j)

            prev = None
            for b in range(BL):
                # ---------- load x + xT (batch 0 pre-issued above) ----------
                x_sb = x_all[b]
                xt_sb = xt_all[b]
                if b > 0:
                    ld_x(b, 0, 0, 2048)
                    ld_x(b, 1, 0, 2048)
                    ld_xt(b, 0, 4096)
                    ld_xt(b, 4096, 8192, nc.gpsimd)
                    ld_x(b, 0, 2048, N)
                    ld_x(b, 1, 2048, N)

                # ---------- phase 1: kT, GT, ksum (software-pipelined), with
                # the previous batch's phase 2 interleaved ----------
                # PSUM zero-region rule: start=True marks the whole 2KB bank
                # pending-zero, so per shared bank only the FIRST accumulation
                # group may start; later groups ride the pending flag.
                gt_ps = ps_gt.tile([128, 512], F32, tag="gt", name=f"gt_{b}")
                ks_ps = ps_ks.tile([128, 512], F32, tag="ks", name=f"ks_{b}")

                def gt_ksum_mms(kt, jpair):
                    for half in range(2):
                        j = 2 * jpair + half
                        ktv = kt[:, half * 256:(half + 1) * 256]
                        for mc in range(2):
                            nc.tensor.matmul(gt_ps[:, mc * 256:(mc + 1) * 256],
                                             lhsT=xt_sb[:, j * 256 + mc * 128:j * 256 + (mc + 1) * 128],
                                             rhs=ktv, start=(j == 0 and mc == 0),
                                             stop=(j == NT - 1),
                                             skip_group_check=True)
                        for cc in range(2):
                            nc.tensor.matmul(ks_ps[:, cc:cc + 1],
                                             lhsT=ktv[:, cc * 128:(cc + 1) * 128],
                                             rhs=onec, start=(j == 0 and cc == 0),
                                             stop=(j == NT - 1),
                                             skip_group_check=True)

                kt_prev = None
                for p in range(NT // 2):
                    ktps = ps_kt.tile([128, 512], F32, tag="kt", name=f"ktps_{b}_{p}")
                    for half in range(2):
                        i = 2 * p + half
                        for k in range(2):
                            nc.tensor.matmul(ktps[:, half * 256:(half + 1) * 256],
                                             lhsT=x_sb[k][:, i * 128:(i + 1) * 128],
                                             rhs=wk[k], start=(half == 0 and k == 0),
                                             stop=(half == 1 and k == 1),
                                             skip_group_check=True)
                    if kt_prev is not None:
                        gt_ksum_mms(*kt_prev)
                    if prev is not None and p % 2 == 1:
                        unit(prev, p // 2)
                    kt_sb = ktp.tile([128, 512], BF16, tag="kt_sb", name=f"kt_sb_{b}_{p}")
                    nc.scalar.activation(kt_sb[:], ktps[:], Relu)
                    kt_prev = (kt_sb, p)
                gt_ksum_mms(*kt_prev)

                # ---------- tail state (DVE) + q tile 0 (fills PE bubble) ----------
                # gt copied in halves so the MT fold pipelines with the copy
                gt_sb = sp.tile([128, 512], BF16, tag="gt_sb", name=f"gt_sb_{b}")
                nc.vector.tensor_copy(gt_sb[:, 0:256], gt_ps[:, 0:256])
                ks_sb = sp.tile([128, 2], F32, tag="ks_sb", name=f"ks_sb_{b}")
                nc.vector.tensor_copy(ks_sb[:], ks_ps[:, 0:2])
                nc.vector.tensor_copy(gt_sb[:, 256:512], gt_ps[:, 256:512])

                q_sb = [qp.tile([128, N], BF16, tag=f"q{mc}", name=f"q{mc}_{b}") for mc in range(2)]
                st_b = {"b": b, "q": q_sb, "yst": None}
                q_mms(st_b, 0)

                # ---------- MT fold + U ----------
                mt_sb = []
                for cs in range(2):
                    mtps = ps_w.tile([128, 512], F32, tag="w", name=f"mtps_{b}_{cs}")
                    for mc in range(2):
                        nc.tensor.matmul(mtps[:, 0:256],
                                         lhsT=gt_sb[:, mc * 256 + cs * 128:mc * 256 + (cs + 1) * 128],
                                         rhs=wvp[mc], start=(mc == 0), stop=(mc == 1))
                    m = sp.tile([128, 256], BF16, tag=f"mt_{cs}", name=f"mt_{b}_{cs}")
                    nc.vector.tensor_copy(m[:], mtps[:, 0:256])
                    mt_sb.append(m)
                u_sb = []
                for cs in range(2):
                    u = sp.tile([128, 128], BF16, tag=f"u_{cs}", name=f"u_{b}_{cs}")
                    nc.vector.tensor_scalar_mul(u[:], ones128, ks_sb[:, cs:cs + 1])
                    u_sb.append(u)

                st_b.update({"mt": mt_sb, "u": u_sb})
                prev = st_b

            # ---------- drain: last batch's q + phase 2, skewed ----------
            prev["drain"] = True
            for j in range(NTW):
                unit(prev, j)
    nc.compile()
    return nc


def _prep_inputs(x, w_qkv, w_proj, bn_gamma, bn_beta, bn_mean, bn_var):
    x = np.asarray(x, dtype=np.float32)
    w_qkv = np.asarray(w_qkv, dtype=np.float32)
    w_proj = np.asarray(w_proj, dtype=np.float32)
    bn_gamma = np.asarray(bn_gamma, dtype=np.float32)
    bn_beta = np.asarray(bn_beta, dtype=np.float32)
    bn_mean = np.asarray(bn_mean, dtype=np.float32)
    bn_var = np.asarray(bn_var, dtype=np.float32)

    # torch-faithful interleave: out-channel 3*i+j -> (channel i, {q,k,v}[j])
    wq_t = w_qkv[0::3].T          # [c_in, c_out]
    wk_t = w_qkv[1::3].T
    wv = w_qkv[2::3]              # [u, c']
    scale = bn_gamma / np.sqrt(bn_var + 1e-5)
    wp_s = scale[:, None] * w_proj            # Wp' [o, u]
    wvp = wv.T @ wp_s.T                       # [c', o]
    bias = bn_beta - bn_mean * scale          # [o]

    cstp = np.zeros((128, CONSTW), dtype=np.float32)
    for k in range(2):
        cstp[:, OFF_WQ + k * 256:OFF_WQ + (k + 1) * 256] = wq_t[k * 128:(k + 1) * 128]
        cstp[:, OFF_WK + k * 256:OFF_WK + (k + 1) * 256] = wk_t[k * 128:(k + 1) * 128]
        cstp[:, OFF_WVP + k * 256:OFF_WVP + (k + 1) * 256] = wvp[k * 128:(k + 1) * 128]
    cstp[:, OFF_ONES:OFF_ONES + 128] = 1.0
    cstp[:, OFF_ONEC] = 1.0
    cst_bf = np.ascontiguousarray(cstp.astype(NPBF16))
    bcol = np.ascontiguousarray(bias.astype(np.float32).reshape(C, 1))

    xf = x.reshape(B, C, N)
    x_bf = np.ascontiguousarray(xf.reshape(B, 2, 128, N).astype(NPBF16))
    # xt[b, p, i*256 + c] = xf[b, c, i*128 + p]
    xt_bf = np.ascontiguousarray(
        xf.reshape(B, C, NT, 128).transpose(0, 3, 2, 1).reshape(B, 128, NT * C)
        .astype(NPBF16))

    in_maps = []
    for core in range(NCORES):
        in_maps.append({
            "x": x_bf[core * BL:(core + 1) * BL],
            "xt": xt_bf[core * BL:(core + 1) * BL],
            "cst": cst_bf,
            "bcol": bcol,
        })
    return in_maps


def _run(inputs, trace=False, **kw):
    if "nc" not in _CACHE:
        _CACHE["nc"] = _build_program()
    nc = _CACHE["nc"]
    in_maps = _prep_inputs(**inputs)
    res = run_bass_kernel_spmd(nc, in_maps, list(range(NCORES)), trace=trace, **kw)
    y = np.concatenate([res.results[i]["y"] for i in range(NCORES)], axis=0)
    return y.reshape(B, C, H, W).astype(np.float32), res


def kernel(**inputs):
    y, _ = _run(inputs)
    return y
